# revision 21
# baseline (speedup 1.0000x reference)
"""Multi-head attention (B=2, S=2048, D=1024, H=16) on 8 NeuronCores.

Sharding: tensor-parallel over heads - 2 heads per core. Each core computes
q/k/v projections for its 128 output columns, full attention for its 2 heads
(both batches), and a partial out-projection [4096, 1024] in bf16. Host sums
the 8 partials (fp64) and adds the output bias.

v3 design: one global software pipeline over 128 "slots" (8 groups x 16 key
tiles). Each slot emits, in PE program order:
  1. the scores pair for (group, kt): both heads as concurrent 64x128
     row-tiled matmuls (contract = head dim = 64; head h lives on SBUF
     partitions 64h..64h+63 in the transposed Q^T/K^T layouts),
  2. one ACT exp over both heads' scores psum [128, 2, 512] (1024
     elems/partition amortizes ACT's ~352-cycle fixed cost) - the ACT engine
     is the attention-phase floor (16.8M exps ~= 147us) and paces the kernel,
  3. the attnV pair trailing 4 slots behind (V|ones trick: psum row 64
     accumulates the softmax denominator),
  4. one "filler" unit popped from a queue: QK-projection half-blocks,
     V-projection key tiles, out-projection row-tiles of finished groups,
     and normalizations - this keeps the PE busy during the ACT-paced
     attention instead of serializing before/after it.

Softmax denominators are inverted per-group with a single [1, 2, 512]
reciprocal_approx_fast (the v1 per-chunk [1,512] nc.vector.reciprocal burned
53us of DVE at 8 cyc/elem and stalled the PE into HAM clock re-throttles),
broadcast once via gpsimd, and applied by two DVE muls straight out of psum.
"""

import os
import numpy as np
import ml_dtypes

B, S, D, H = 2, 2048, 1024, 16
HD = D // H          # 64
BS = B * S           # 4096 tokens
NCORES = 8
HPC = H // NCORES    # heads per core = 2
CPC = HPC * HD       # output cols per core = 128
KC = D // 128        # contract chunks = 8
NKT = S // 128       # 16 key tiles per batch
QG = 512             # q-group width (one psum bank of fp32)
NQG = S // QG        # 4 q-groups per batch
TRAIL = 4            # attnV trails scores by this many slots

BF16 = ml_dtypes.bfloat16

_prog = None


def _build_program():
    import concourse.bacc as bacc
    import concourse.tile as tile
    from concourse import mybir

    f32 = mybir.dt.float32
    bf16 = mybir.dt.bfloat16
    AF = mybir.ActivationFunctionType

    nc = bacc.Bacc("TRN2", debug=False, enable_asserts=False, num_devices=NCORES)

    xT = nc.dram_tensor("xT", [D, BS], bf16, kind="ExternalInput").ap()
    wq = nc.dram_tensor("wq", [D, CPC], bf16, kind="ExternalInput").ap()
    wk = nc.dram_tensor("wk", [D, CPC], bf16, kind="ExternalInput").ap()
    wv = nc.dram_tensor("wv", [D, CPC], bf16, kind="ExternalInput").ap()
    wo = nc.dram_tensor("wo", [CPC, D], bf16, kind="ExternalInput").ap()
    bq = nc.dram_tensor("bq", [CPC, 1], f32, kind="ExternalInput").ap()
    bk = nc.dram_tensor("bk", [CPC, 1], f32, kind="ExternalInput").ap()
    bv = nc.dram_tensor("bv", [1, CPC], bf16, kind="ExternalInput").ap()
    out = nc.dram_tensor("out", [BS, D], bf16, kind="ExternalOutput").ap()

    SCALE = float(1.0 / np.sqrt(HD))

    with tile.TileContext(nc) as tc:
        with (
            tc.tile_pool(name="big", bufs=1) as big,
            tc.tile_pool(name="sm", bufs=1) as sm,
            tc.tile_pool(name="attn", bufs=2) as attn,
            tc.tile_pool(name="etp", bufs=8) as etp,
            tc.tile_pool(name="nrm", bufs=2) as nrm,
            tc.tile_pool(name="ostage", bufs=3) as ostage,
            tc.tile_pool(name="ps", bufs=2, space="PSUM") as ps,
        ):
            # ---- resident SBUF tensors ----
            xt_sb = big.tile([128, KC, BS], bf16, name="xt_sb", tag="xt")
            qt_sb = big.tile([128, BS], bf16, name="qt_sb", tag="qt")
            kt_sb = big.tile([128, BS], bf16, name="kt_sb", tag="kt")
            # V|ones per head: [keys(128) x keytile(32) x (64 V + 1 ones)*2]
            v_sb = big.tile([128, B * NKT, 2 * (HD + 1)], bf16, name="v_sb", tag="v")
            wo_sb = big.tile([128, D], bf16, name="wo_sb", tag="wo")

            wq_sb = sm.tile([128, KC, CPC], bf16, name="wq_sb", tag="wq")
            wk_sb = sm.tile([128, KC, CPC], bf16, name="wk_sb", tag="wk")
            wv_sb = sm.tile([128, KC, CPC], bf16, name="wv_sb", tag="wv")
            bq_sb = sm.tile([CPC, 1], f32, name="bq_sb", tag="bq")
            bk_sb = sm.tile([CPC, 1], f32, name="bk_sb", tag="bk")
            bv_sb = sm.tile([1, CPC], bf16, name="bv_sb", tag="bv")
            ones_bf = sm.tile([1, 128], bf16, name="ones_bf", tag="onesb")

            nc.vector.memset(ones_bf, 1.0)
            nc.vector.memset(v_sb[:, :, HD : HD + 1], 1.0)
            nc.vector.memset(v_sb[:, :, 2 * HD + 1 : 2 * HD + 2], 1.0)

            # DMA order: first QK-proj unit needs wq/wk + tokens 0:512 only
            xt_r = xT.rearrange("(c p) n -> p c n", p=128)
            nc.sync.dma_start(out=wq_sb, in_=wq.rearrange("(c p) n -> p c n", p=128))
            nc.sync.dma_start(out=wk_sb, in_=wk.rearrange("(c p) n -> p c n", p=128))
            for c in range(KC):
                nc.sync.dma_start(out=xt_sb[:, c, 0:512], in_=xt_r[:, c, 0:512])
            nc.sync.dma_start(out=bq_sb, in_=bq)
            nc.sync.dma_start(out=bk_sb, in_=bk)
            for c in range(KC):
                nc.sync.dma_start(out=xt_sb[:, c, 512:1024], in_=xt_r[:, c, 512:1024])
            for c in range(KC):
                nc.sync.dma_start(out=xt_sb[:, c, 1024:2048], in_=xt_r[:, c, 1024:2048])
            nc.sync.dma_start(out=wv_sb, in_=wv.rearrange("(c p) n -> p c n", p=128))
            nc.sync.dma_start(out=bv_sb, in_=bv)
            for tb in range(2, BS // 1024):
                for c in range(KC):
                    nc.sync.dma_start(
                        out=xt_sb[:, c, tb * 1024 : (tb + 1) * 1024],
                        in_=xt_r[:, c, tb * 1024 : (tb + 1) * 1024],
                    )
            nc.sync.dma_start(out=wo_sb, in_=wo)

            groups = [(b, qg) for b in range(B) for qg in range(NQG)]
            NG = len(groups)

            et_tiles = {}   # (gi, kt) -> et tile [128, 2, QG]
            op_tiles = {}   # (gi, h) -> attnV psum tile [65, QG]
            ot_tiles = {}   # b -> ot_sb [128, S]

            # ---- filler units (each ~1-2us of PE work) ----
            def qk_unit(is_q, half):
                # projects Q^T or K^T for tokens [512*half, 512*(half+1))
                w_sb, b_sb, dst = (
                    (wq_sb, bq_sb, qt_sb) if is_q else (wk_sb, bk_sb, kt_sb)
                )
                tok = half * 512

                def run():
                    pp = ps.tile(
                        [128, 512], f32, name=f"pp{int(is_q)}_{half}", tag="op", bufs=4
                    )
                    for c in range(KC):
                        nc.tensor.matmul(
                            pp,
                            lhsT=w_sb[:, c, :],
                            rhs=xt_sb[:, c, tok : tok + 512],
                            start=(c == 0),
                            stop=(c == KC - 1),
                        )
                    nc.vector.tensor_scalar_add(dst[:, tok : tok + 512], pp, b_sb)

                return run

            def v_unit(kt):
                # V projection (natural [keys, cols] layout) for one key
                # tile; bias via rank-1 ones matmul into the accumulation
                def run():
                    vp = ps.tile([128, CPC], f32, name=f"vp{kt}", tag="op", bufs=4)
                    for c in range(KC):
                        nc.tensor.matmul(
                            vp,
                            lhsT=xt_sb[:, c, kt * 128 : (kt + 1) * 128],
                            rhs=wv_sb[:, c, :],
                            start=(c == 0),
                            stop=False,
                        )
                    nc.tensor.matmul(
                        vp, lhsT=ones_bf, rhs=bv_sb, start=False, stop=True
                    )
                    nc.vector.tensor_copy(
                        v_sb[:, kt, :].rearrange("p (h c) -> p h c", h=2)[:, :, 0:HD],
                        vp.rearrange("p (h c) -> p h c", h=2),
                    )

                return run

            stag_tiles = {}  # gi -> [64, 2, QG] bf16 unnormalized o
            den_tiles = {}   # gi -> [1, 2, QG] f32 softmax denominators

            def evac_group(gi):
                # free the attnV psum accumulators ASAP: pull the denominator
                # row (f32) and the unnormalized output (bf16) into SBUF so
                # the next groups' accumulators never wait on the (slow)
                # normalization chain
                den = nrm.tile([1, 2, QG], f32, name=f"den{gi}", tag="den")
                stag = nrm.tile([HD, 2, QG], bf16, name=f"stag{gi}", tag="stag")
                for h in range(HPC):
                    op = op_tiles.pop((gi, h))
                    nc.vector.tensor_copy(den[0:1, h, :], op[HD : HD + 1, :])
                    nc.vector.tensor_copy(stag[:, h, :], op[0:HD, :])
                den_tiles[gi] = den
                stag_tiles[gi] = stag

            def norm_unit(gi):
                b, qg = groups[gi]

                def run():
                    if qg == 0:
                        ot_tiles[b] = attn.tile(
                            [128, S], bf16, name=f"ot{b}", tag="ot"
                        )
                    ot = ot_tiles[b]
                    den = den_tiles.pop(gi)
                    stag = stag_tiles.pop(gi)
                    rq = nrm.tile([1, 2, QG], f32, name=f"rq{gi}", tag="rq")
                    nc.vector.reciprocal_approx_fast(rq, den)
                    rqb = nrm.tile([1, 2, QG], bf16, name=f"rqb{gi}", tag="rqb")
                    nc.vector.tensor_copy(rqb, rq)
                    rbs = nrm.tile([HD, 2, QG], bf16, name=f"rbs{gi}", tag="rbs")
                    nc.gpsimd.partition_broadcast(rbs, rqb)
                    for h in range(HPC):
                        nc.vector.tensor_mul(
                            ot[h * HD : (h + 1) * HD, qg * QG : (qg + 1) * QG],
                            stag[:, h, :],
                            rbs[:, h, :],
                        )

                return run

            def oproj_unit(gi, qt):
                b, qg = groups[gi]

                def run():
                    ot = ot_tiles[b]
                    os_ = ostage.tile([128, 1024], bf16, name=f"os{gi}_{qt}", tag="os")
                    pq = ps.tile([128, 1024], f32, name=f"pq{gi}{qt}", tag="sp")
                    for nh in range(2):
                        nc.tensor.matmul(
                            pq[:, nh * 512 : (nh + 1) * 512],
                            lhsT=ot[:, qt * 128 : (qt + 1) * 128],
                            rhs=wo_sb[:, nh * 512 : (nh + 1) * 512],
                            start=True,
                            stop=True,
                        )
                    nc.vector.tensor_copy(os_, pq)
                    nc.sync.dma_start(
                        out=out[b * S + qt * 128 : b * S + (qt + 1) * 128, :],
                        in_=os_,
                    )

                return run

            # ---- pipeline stages ----
            def scores_pair(gi, kt):
                b, qg = groups[gi]
                q0 = b * S + qg * QG
                k0 = b * S + kt * 128
                sp = ps.tile([128, 2, QG], f32, name=f"sp{gi}_{kt}", tag="sp")
                for h in range(HPC):
                    hp = h * HD
                    nc.tensor.matmul(
                        sp[:, h, :],
                        lhsT=kt_sb[hp : hp + HD, k0 : k0 + 128],
                        rhs=qt_sb[hp : hp + HD, q0 : q0 + QG],
                        start=True,
                        stop=True,
                        tile_position=(hp, 0),
                    )
                et = etp.tile([128, 2, QG], bf16, name=f"et{gi}_{kt}", tag="et")
                nc.scalar.activation(et, sp, AF.Exp, scale=SCALE)
                et_tiles[(gi, kt)] = et

            def attnv_pair(gi, kt):
                b, qg = groups[gi]
                et = et_tiles.pop((gi, kt))
                for h in range(HPC):
                    if kt == 0:
                        op_tiles[(gi, h)] = ps.tile(
                            [HD + 1, QG], f32, name=f"op{gi}_{h}", tag="op", bufs=4
                        )
                    nc.tensor.matmul(
                        op_tiles[(gi, h)],
                        lhsT=v_sb[:, b * NKT + kt, h * (HD + 1) : (h + 1) * (HD + 1)],
                        rhs=et[:, h, :],
                        start=(kt == 0),
                        stop=(kt == NKT - 1),
                    )

            # ---- the slot machine ----
            from collections import deque

            fillers = deque()
            # b0 K halves 2,3 (keys kt8-15, needed by slot 8)
            fillers.append(qk_unit(False, 2))
            fillers.append(qk_unit(False, 3))
            for kt in range(NKT):             # b0 V (attnV g0 from slot 4)
                fillers.append(v_unit(kt))
            fillers.append(qk_unit(True, 1))  # q for g1 (slot 16)
            fillers.append(qk_unit(True, 2))  # q for g2 (slot 32)
            fillers.append(qk_unit(True, 3))  # q for g3 (slot 48)
            for half in range(4, 8):          # b1 keys (slot 64)
                fillers.append(qk_unit(False, half))
            fillers.append(qk_unit(True, 4))  # q for g4 (slot 64)
            for kt in range(NKT, 2 * NKT):    # b1 V (attnV g4 from slot 68)
                fillers.append(v_unit(kt))
            fillers.append(qk_unit(True, 5))
            fillers.append(qk_unit(True, 6))
            fillers.append(qk_unit(True, 7))

            # lead-in: q/k needed by group 0's first scores
            qk_unit(True, 0)()
            qk_unit(False, 0)()
            qk_unit(False, 1)()

            pending_attnv = deque()
            tail_reserve = []

            def emit_rest(slot):
                while pending_attnv and pending_attnv[0][0] <= slot:
                    _, agi, akt = pending_attnv.popleft()
                    attnv_pair(agi, akt)
                    if akt == NKT - 1:
                        evac_group(agi)
                        b, qg = groups[agi]
                        qts = list(range(qg * (QG // 128), (qg + 1) * (QG // 128)))
                        if agi == NG - 2:
                            # hold the last two out-proj row-tiles back: they
                            # keep the PE busy (and the HAM clock warm) while
                            # the last group's normalization chain runs
                            tail_reserve.extend(oproj_unit(agi, qt) for qt in qts[2:])
                            qts = qts[:2]
                        fillers.appendleft(norm_unit(agi))
                        for qt in reversed(qts):
                            fillers.insert(1, oproj_unit(agi, qt))
                        if agi == NG - 1:
                            for i, u in enumerate(tail_reserve):
                                fillers.insert(1 + i, u)
                npop = 2 if slot < 2 * NKT else 1
                for _ in range(npop):
                    if fillers:
                        fillers.popleft()()

            for slot in range(NG * NKT):
                gi, kt = divmod(slot, NKT)
                scores_pair(gi, kt)
                trail = 2 if gi == NG - 1 else TRAIL
                pending_attnv.append((slot + trail, gi, kt))
                emit_rest(slot)
            slot = NG * NKT
            while pending_attnv or fillers:
                emit_rest(slot)
                slot += 1

    nc.compile()
    return nc


def _get_prog():
    global _prog
    if _prog is None:
        _prog = _build_program()
    return _prog


def kernel(x, Wq, bq, Wk, bk, Wv, bv, Wo, bo):
    from concourse import bass_utils

    nc = _get_prog()

    xT = np.ascontiguousarray(
        np.asarray(x, dtype=np.float32).reshape(BS, D).T
    ).astype(BF16)

    in_maps = []
    for c in range(NCORES):
        cols = slice(c * CPC, (c + 1) * CPC)
        in_maps.append(
            {
                "xT": xT,
                "wq": np.ascontiguousarray(Wq[cols, :].T).astype(BF16),
                "wk": np.ascontiguousarray(Wk[cols, :].T).astype(BF16),
                "wv": np.ascontiguousarray(Wv[cols, :].T).astype(BF16),
                "wo": np.ascontiguousarray(Wo[:, cols].T).astype(BF16),
                "bq": np.asarray(bq[cols], np.float32).reshape(CPC, 1),
                "bk": np.asarray(bk[cols], np.float32).reshape(CPC, 1),
                "bv": np.asarray(bv[cols], np.float32).reshape(1, CPC).astype(BF16),
            }
        )

    res = bass_utils.run_bass_kernel_spmd(
        nc,
        in_maps,
        core_ids=list(range(NCORES)),
        trace=bool(int(os.environ.get("KERNEL_TRACE", "0"))),
    )
    kernel.last_results = res

    acc = np.zeros((BS, D), np.float64)
    for c in range(NCORES):
        acc += res.results[c]["out"].astype(np.float64)
    acc += np.asarray(bo, np.float64)[None, :]
    return acc.reshape(B, S, D).astype(np.float32)


# revision 23
# speedup vs baseline: 1.1745x; 1.1745x over previous
"""Multi-head attention (B=2, S=2048, D=1024, H=16) on 8 NeuronCores.

Sharding: tensor-parallel over heads - 2 heads per core. Each core computes
q/k/v projections for its 128 output columns, full attention for its 2 heads
(both batches), and a partial out-projection [4096, 1024] in bf16. Host sums
the 8 partials (fp64) and adds the output bias.

v3 design: one global software pipeline over 128 "slots" (8 groups x 16 key
tiles). Each slot emits, in PE program order:
  1. the scores pair for (group, kt): both heads as concurrent 64x128
     row-tiled matmuls (contract = head dim = 64; head h lives on SBUF
     partitions 64h..64h+63 in the transposed Q^T/K^T layouts),
  2. one ACT exp over both heads' scores psum [128, 2, 512] (1024
     elems/partition amortizes ACT's ~352-cycle fixed cost) - the ACT engine
     is the attention-phase floor (16.8M exps ~= 147us) and paces the kernel,
  3. the attnV pair trailing 4 slots behind (V|ones trick: psum row 64
     accumulates the softmax denominator),
  4. one "filler" unit popped from a queue: QK-projection half-blocks,
     V-projection key tiles, out-projection row-tiles of finished groups,
     and normalizations - this keeps the PE busy during the ACT-paced
     attention instead of serializing before/after it.

Softmax denominators are inverted per-group with a single [1, 2, 512]
reciprocal_approx_fast (the v1 per-chunk [1,512] nc.vector.reciprocal burned
53us of DVE at 8 cyc/elem and stalled the PE into HAM clock re-throttles),
broadcast once via gpsimd, and applied by two DVE muls straight out of psum.
"""

import os
import numpy as np
import ml_dtypes

B, S, D, H = 2, 2048, 1024, 16
HD = D // H          # 64
BS = B * S           # 4096 tokens
NCORES = 8
HPC = H // NCORES    # heads per core = 2
CPC = HPC * HD       # output cols per core = 128
KC = D // 128        # contract chunks = 8
NKT = S // 128       # 16 key tiles per batch
QG = 512             # q-group width (one psum bank of fp32)
NQG = S // QG        # 4 q-groups per batch
TRAIL = 4            # attnV trails scores by this many slots

BF16 = ml_dtypes.bfloat16

_prog = None


def _build_program():
    import concourse.bacc as bacc
    import concourse.tile as tile
    from concourse import mybir

    f32 = mybir.dt.float32
    bf16 = mybir.dt.bfloat16
    AF = mybir.ActivationFunctionType

    nc = bacc.Bacc("TRN2", debug=False, enable_asserts=False, num_devices=NCORES)

    xT = nc.dram_tensor("xT", [D, BS], bf16, kind="ExternalInput").ap()
    wq = nc.dram_tensor("wq", [D, CPC], bf16, kind="ExternalInput").ap()
    wk = nc.dram_tensor("wk", [D, CPC], bf16, kind="ExternalInput").ap()
    wv = nc.dram_tensor("wv", [D, CPC], bf16, kind="ExternalInput").ap()
    wo = nc.dram_tensor("wo", [CPC, D], bf16, kind="ExternalInput").ap()
    bq = nc.dram_tensor("bq", [CPC, 1], f32, kind="ExternalInput").ap()
    bk = nc.dram_tensor("bk", [CPC, 1], f32, kind="ExternalInput").ap()
    bv = nc.dram_tensor("bv", [1, CPC], bf16, kind="ExternalInput").ap()
    out = nc.dram_tensor("out", [BS, D], bf16, kind="ExternalOutput").ap()

    SCALE = float(1.0 / np.sqrt(HD))

    with tile.TileContext(nc) as tc:
        with (
            tc.tile_pool(name="big", bufs=1) as big,
            tc.tile_pool(name="sm", bufs=1) as sm,
            tc.tile_pool(name="attn", bufs=2) as attn,
            tc.tile_pool(name="etp", bufs=8) as etp,
            tc.tile_pool(name="nrm", bufs=2) as nrm,
            tc.tile_pool(name="ostage", bufs=3) as ostage,
            tc.tile_pool(name="ps", bufs=2, space="PSUM") as ps,
        ):
            # ---- resident SBUF tensors ----
            xt_sb = big.tile([128, KC, BS], bf16, name="xt_sb", tag="xt")
            qt_sb = big.tile([128, BS], bf16, name="qt_sb", tag="qt")
            kt_sb = big.tile([128, BS], bf16, name="kt_sb", tag="kt")
            # V|ones per head: [keys(128) x keytile(32) x (64 V + 1 ones)*2]
            v_sb = big.tile([128, B * NKT, 2 * (HD + 1)], bf16, name="v_sb", tag="v")
            wo_sb = big.tile([128, D], bf16, name="wo_sb", tag="wo")

            wq_sb = sm.tile([128, KC, CPC], bf16, name="wq_sb", tag="wq")
            wk_sb = sm.tile([128, KC, CPC], bf16, name="wk_sb", tag="wk")
            wv_sb = sm.tile([128, KC, CPC], bf16, name="wv_sb", tag="wv")
            bq_sb = sm.tile([CPC, 1], f32, name="bq_sb", tag="bq")
            bk_sb = sm.tile([CPC, 1], f32, name="bk_sb", tag="bk")
            bv_sb = sm.tile([1, CPC], bf16, name="bv_sb", tag="bv")
            ones_bf = sm.tile([1, 128], bf16, name="ones_bf", tag="onesb")

            nc.vector.memset(ones_bf, 1.0)
            nc.vector.memset(v_sb[:, :, HD : HD + 1], 1.0)
            nc.vector.memset(v_sb[:, :, 2 * HD + 1 : 2 * HD + 2], 1.0)

            # DMA order: first QK-proj unit needs wq/wk + tokens 0:512 only
            xt_r = xT.rearrange("(c p) n -> p c n", p=128)
            nc.sync.dma_start(out=wq_sb, in_=wq.rearrange("(c p) n -> p c n", p=128))
            nc.sync.dma_start(out=wk_sb, in_=wk.rearrange("(c p) n -> p c n", p=128))
            for c in range(KC):
                nc.sync.dma_start(out=xt_sb[:, c, 0:512], in_=xt_r[:, c, 0:512])
            nc.sync.dma_start(out=bq_sb, in_=bq)
            nc.sync.dma_start(out=bk_sb, in_=bk)
            for c in range(KC):
                nc.sync.dma_start(out=xt_sb[:, c, 512:1024], in_=xt_r[:, c, 512:1024])
            for c in range(KC):
                nc.sync.dma_start(out=xt_sb[:, c, 1024:2048], in_=xt_r[:, c, 1024:2048])
            nc.sync.dma_start(out=wv_sb, in_=wv.rearrange("(c p) n -> p c n", p=128))
            nc.sync.dma_start(out=bv_sb, in_=bv)
            for tb in range(2, BS // 1024):
                for c in range(KC):
                    nc.sync.dma_start(
                        out=xt_sb[:, c, tb * 1024 : (tb + 1) * 1024],
                        in_=xt_r[:, c, tb * 1024 : (tb + 1) * 1024],
                    )
            nc.sync.dma_start(out=wo_sb, in_=wo)

            groups = [(b, qg) for b in range(B) for qg in range(NQG)]
            NG = len(groups)

            et_tiles = {}   # (gi, kt) -> et tile [128, 2, QG]
            op_tiles = {}   # (gi, h) -> attnV psum tile [65, QG]
            ot_tiles = {}   # b -> ot_sb [128, S]

            # ---- filler units (each ~1-2us of PE work) ----
            def qk_unit(is_q, half):
                # projects Q^T or K^T for tokens [512*half, 512*(half+1))
                w_sb, b_sb, dst = (
                    (wq_sb, bq_sb, qt_sb) if is_q else (wk_sb, bk_sb, kt_sb)
                )
                tok = half * 512

                def run():
                    pp = ps.tile(
                        [128, 512], f32, name=f"pp{int(is_q)}_{half}", tag="op", bufs=4
                    )
                    for c in range(KC):
                        nc.tensor.matmul(
                            pp,
                            lhsT=w_sb[:, c, :],
                            rhs=xt_sb[:, c, tok : tok + 512],
                            start=(c == 0),
                            stop=(c == KC - 1),
                        )
                    nc.vector.tensor_scalar_add(dst[:, tok : tok + 512], pp, b_sb)

                return run

            def v_unit(kt):
                # V projection (natural [keys, cols] layout) for one key
                # tile; bias via rank-1 ones matmul into the accumulation
                def run():
                    vp = ps.tile([128, CPC], f32, name=f"vp{kt}", tag="op", bufs=4)
                    for c in range(KC):
                        nc.tensor.matmul(
                            vp,
                            lhsT=xt_sb[:, c, kt * 128 : (kt + 1) * 128],
                            rhs=wv_sb[:, c, :],
                            start=(c == 0),
                            stop=False,
                        )
                    nc.tensor.matmul(
                        vp, lhsT=ones_bf, rhs=bv_sb, start=False, stop=True
                    )
                    nc.vector.tensor_copy(
                        v_sb[:, kt, :].rearrange("p (h c) -> p h c", h=2)[:, :, 0:HD],
                        vp.rearrange("p (h c) -> p h c", h=2),
                    )

                return run

            stag_tiles = {}  # gi -> [64, 2, QG] bf16 unnormalized o
            den_tiles = {}   # gi -> [1, 2, QG] f32 softmax denominators

            def evac_group(gi):
                # free the attnV psum accumulators ASAP: pull the denominator
                # row (f32) and the unnormalized output (bf16) into SBUF so
                # the next groups' accumulators never wait on the (slow)
                # normalization chain
                den = nrm.tile([1, 2, QG], f32, name=f"den{gi}", tag="den")
                stag = nrm.tile([HD, 2, QG], bf16, name=f"stag{gi}", tag="stag")
                for h in range(HPC):
                    op = op_tiles.pop((gi, h))
                    nc.vector.tensor_copy(den[0:1, h, :], op[HD : HD + 1, :])
                    nc.vector.tensor_copy(stag[:, h, :], op[0:HD, :])
                den_tiles[gi] = den
                stag_tiles[gi] = stag

            def norm_unit(gi):
                b, qg = groups[gi]

                def run():
                    if qg == 0:
                        ot_tiles[b] = attn.tile(
                            [128, S], bf16, name=f"ot{b}", tag="ot"
                        )
                    ot = ot_tiles[b]
                    den = den_tiles.pop(gi)
                    stag = stag_tiles.pop(gi)
                    rq = nrm.tile([1, 2, QG], f32, name=f"rq{gi}", tag="rq")
                    nc.vector.reciprocal_approx_fast(rq, den)
                    rqb = nrm.tile([1, 2, QG], bf16, name=f"rqb{gi}", tag="rqb")
                    nc.vector.tensor_copy(rqb, rq)
                    rbs = nrm.tile([HD, 2, QG], bf16, name=f"rbs{gi}", tag="rbs")
                    nc.gpsimd.partition_broadcast(rbs, rqb)
                    for h in range(HPC):
                        nc.vector.tensor_mul(
                            ot[h * HD : (h + 1) * HD, qg * QG : (qg + 1) * QG],
                            stag[:, h, :],
                            rbs[:, h, :],
                        )

                return run

            def oproj_unit(gi, qt):
                b, qg = groups[gi]

                def run():
                    ot = ot_tiles[b]
                    os_ = ostage.tile([128, 1024], bf16, name=f"os{gi}_{qt}", tag="os")
                    for nh in range(2):
                        pq = ps.tile(
                            [128, 512], f32, name=f"pq{gi}{qt}{nh}", tag="op", bufs=4
                        )
                        nc.tensor.matmul(
                            pq,
                            lhsT=ot[:, qt * 128 : (qt + 1) * 128],
                            rhs=wo_sb[:, nh * 512 : (nh + 1) * 512],
                            start=True,
                            stop=True,
                        )
                        nc.vector.tensor_copy(os_[:, nh * 512 : (nh + 1) * 512], pq)
                    nc.sync.dma_start(
                        out=out[b * S + qt * 128 : b * S + (qt + 1) * 128, :],
                        in_=os_,
                    )

                return run

            # ---- pipeline stages ----
            def scores_pair(gi, kt):
                b, qg = groups[gi]
                q0 = b * S + qg * QG
                k0 = b * S + kt * 128
                sp = ps.tile([128, 2, QG], f32, name=f"sp{gi}_{kt}", tag="sp")
                for h in range(HPC):
                    hp = h * HD
                    nc.tensor.matmul(
                        sp[:, h, :],
                        lhsT=kt_sb[hp : hp + HD, k0 : k0 + 128],
                        rhs=qt_sb[hp : hp + HD, q0 : q0 + QG],
                        start=True,
                        stop=True,
                        tile_position=(hp, 0),
                    )
                et = etp.tile([128, 2, QG], bf16, name=f"et{gi}_{kt}", tag="et")
                nc.scalar.activation(et, sp, AF.Exp, scale=SCALE)
                et_tiles[(gi, kt)] = et

            def attnv_pair(gi, kt):
                b, qg = groups[gi]
                et = et_tiles.pop((gi, kt))
                for h in range(HPC):
                    if kt == 0:
                        op_tiles[(gi, h)] = ps.tile(
                            [HD + 1, QG], f32, name=f"op{gi}_{h}", tag="op", bufs=4
                        )
                    nc.tensor.matmul(
                        op_tiles[(gi, h)],
                        lhsT=v_sb[:, b * NKT + kt, h * (HD + 1) : (h + 1) * (HD + 1)],
                        rhs=et[:, h, :],
                        start=(kt == 0),
                        stop=(kt == NKT - 1),
                    )

            # ---- the slot machine ----
            from collections import deque

            fillers = deque()
            # b0 K halves 2,3 (keys kt8-15, needed by slot 8)
            fillers.append(qk_unit(False, 2))
            fillers.append(qk_unit(False, 3))
            for kt in range(NKT):             # b0 V (attnV g0 from slot 4)
                fillers.append(v_unit(kt))
            fillers.append(qk_unit(True, 1))  # q for g1 (slot 16)
            fillers.append(qk_unit(True, 2))  # q for g2 (slot 32)
            fillers.append(qk_unit(True, 3))  # q for g3 (slot 48)
            for half in range(4, 8):          # b1 keys (slot 64)
                fillers.append(qk_unit(False, half))
            fillers.append(qk_unit(True, 4))  # q for g4 (slot 64)
            for kt in range(NKT, 2 * NKT):    # b1 V (attnV g4 from slot 68)
                fillers.append(v_unit(kt))
            fillers.append(qk_unit(True, 5))
            fillers.append(qk_unit(True, 6))
            fillers.append(qk_unit(True, 7))

            # lead-in: q/k needed by group 0's first scores
            qk_unit(True, 0)()
            qk_unit(False, 0)()
            qk_unit(False, 1)()

            import heapq

            scheduled = []  # heap of (target_slot, seq, fn)
            seq_ctr = [0]

            def schedule(target, fn):
                heapq.heappush(scheduled, (target, seq_ctr[0], fn))
                seq_ctr[0] += 1

            def make_attnv(gi, kt):
                def run_at(slot):
                    attnv_pair(gi, kt)
                    if kt == NKT - 1:
                        evac_group(gi)
                        b, qg = groups[gi]
                        qts = list(range(qg * (QG // 128), (qg + 1) * (QG // 128)))
                        schedule(slot + 1, lambda s: norm_unit(gi)())
                        if gi == NG - 2:
                            # hold the last two out-proj row-tiles back: they
                            # keep the PE busy (and the HAM clock warm) while
                            # the last group's normalization chain runs
                            tail = NG * NKT + 3
                            for j, qt in enumerate(qts[2:]):
                                schedule(tail + j, (lambda q: lambda s: oproj_unit(gi, q)())(qt))
                            qts = qts[:2]
                        # ot(gi) is ready ~4-5 slots after the norm chain
                        # starts; emitting out-proj earlier stalls the
                        # in-order PE stream on the DVE normalization
                        for j, qt in enumerate(qts):
                            schedule(slot + 5 + j, (lambda q: lambda s: oproj_unit(gi, q)())(qt))

                return run_at

            def emit_rest(slot):
                ran = 0
                while scheduled and scheduled[0][0] <= slot:
                    _, _, fn = heapq.heappop(scheduled)
                    fn(slot)
                    ran += 1
                npop = 2 if slot < 2 * NKT else 1
                for _ in range(npop):
                    if fillers:
                        fillers.popleft()()

            for slot in range(NG * NKT):
                gi, kt = divmod(slot, NKT)
                scores_pair(gi, kt)
                trail = 2 if gi == NG - 1 else TRAIL
                schedule(slot + trail, make_attnv(gi, kt))
                emit_rest(slot)
            slot = NG * NKT
            while scheduled or fillers:
                emit_rest(slot)
                slot += 1

    nc.compile()
    return nc


def _get_prog():
    global _prog
    if _prog is None:
        _prog = _build_program()
    return _prog


def kernel(x, Wq, bq, Wk, bk, Wv, bv, Wo, bo):
    from concourse import bass_utils

    nc = _get_prog()

    xT = np.ascontiguousarray(
        np.asarray(x, dtype=np.float32).reshape(BS, D).T
    ).astype(BF16)

    in_maps = []
    for c in range(NCORES):
        cols = slice(c * CPC, (c + 1) * CPC)
        in_maps.append(
            {
                "xT": xT,
                "wq": np.ascontiguousarray(Wq[cols, :].T).astype(BF16),
                "wk": np.ascontiguousarray(Wk[cols, :].T).astype(BF16),
                "wv": np.ascontiguousarray(Wv[cols, :].T).astype(BF16),
                "wo": np.ascontiguousarray(Wo[:, cols].T).astype(BF16),
                "bq": np.asarray(bq[cols], np.float32).reshape(CPC, 1),
                "bk": np.asarray(bk[cols], np.float32).reshape(CPC, 1),
                "bv": np.asarray(bv[cols], np.float32).reshape(1, CPC).astype(BF16),
            }
        )

    res = bass_utils.run_bass_kernel_spmd(
        nc,
        in_maps,
        core_ids=list(range(NCORES)),
        trace=bool(int(os.environ.get("KERNEL_TRACE", "0"))),
    )
    kernel.last_results = res

    acc = np.zeros((BS, D), np.float64)
    for c in range(NCORES):
        acc += res.results[c]["out"].astype(np.float64)
    acc += np.asarray(bo, np.float64)[None, :]
    return acc.reshape(B, S, D).astype(np.float32)
